# revision 3
# baseline (speedup 1.0000x reference)
"""1D horizontal correlation (FlowNet cost volume, kernel_size=1) on 8 TRN2 cores.

out[b, d+4, y, x] = mean_c x1[b,c,y,x] * x2[b,c,y,x+d],  d in [-4, 4], OOB -> 0

v2 strategy (vs the v1 band-to-host baseline):
- Data-parallel over batch: B=8 -> one batch element per NeuronCore.
- Per core, flatten (H, W) -> S=30720 positions. C=128 = partition dim.
- Band matmuls as before: psum[m, n] = sum_c x1[c,128t+m] x2[c,128t-4+n].
  3 matmuls share one PSUM bank ([128, 3*136] fp32) so one wide ACT/DVE
  copy drains 3 tiles at once (v1's 1:1 copy-per-matmul serialized the
  whole pipeline at ~298ns/tile).
- Per chunk of K tiles, a gpsimd indirect_copy gathers, for each
  16-partition group g, the 24-wide window band[p, 136k+16g : +24]
  (3 idx blocks of 8 elems per tile; indices shared within a group).
  This shrinks the output from the full 136-band (8.35MB) to 1.47MB.
- Xc chunks DMA to DRAM contiguously; the host applies the remaining
  per-partition fine shear Y[p,t,j] = Xc[p,t,(p%16)+j] with 16 strided
  slice copies (no elementwise gather) and the OOB zero mask.
- Input DMAs are issued on SP (v1 issued them on ACT, stalling the copy
  engine ~21us); ACT+DVE alternate psum-group copies; Pool does gathers.
"""

import os
import numpy as np

import concourse.bass as bass
import concourse.bacc as bacc
import concourse.mybir as mybir
import concourse.tile as tile
from concourse import bass_utils

B, C, H, W = 8, 128, 96, 320
S = H * W            # 30720 positions per batch element
MAXD = 4
ND = 2 * MAXD + 1    # 9 displacement channels
TP = 128             # positions per tile (PSUM partition dim)
NT = S // TP         # 240 tiles
NB = TP + 2 * MAXD   # 136 band columns per tile
GRP = 3              # matmuls packed per PSUM bank (3*136*4B = 1632B < 2KB)
# (tiles, psum-group size) per chunk: the small GRP=1 tail chunk shortens the
# final drain (185ns single-tile copies instead of one 525ns triple copy)
CHUNKS = [(33, 3)] * 7 + [(9, 1)]
WIN = 24             # gathered window width per 16-partition group


def _chunk_idx_cols(k):
    return (3 * k + 15) // 16


NIDX_COLS = sum(_chunk_idx_cols(k) for k, _ in sorted(set(CHUNKS)))  # 7 + 2
# input DMA slices: full-width early, finer at the end so the last matmuls
# are not gated by one big 1920-col transfer
SLICES = [1920] * 15 + [640, 640, 640]
assert sum(SLICES) == S and sum(k for k, _ in CHUNKS) == NT

F32 = mybir.dt.float32
BF16 = mybir.dt.bfloat16
U16 = mybir.dt.uint16


def _build_nc():
    nc = bacc.Bacc(debug=False)
    x1 = nc.dram_tensor("x1", [C, S], BF16, kind="ExternalInput")
    # x2 host-padded with a zero halo of MAXD on both ends: col j = pos j - 4
    x2 = nc.dram_tensor("x2", [C, S + 2 * MAXD], BF16, kind="ExternalInput")
    cidx = nc.dram_tensor("cidx", [C, NIDX_COLS], U16, kind="ExternalInput")
    xc = nc.dram_tensor("xc", [C, NT * WIN], BF16, kind="ExternalOutput")

    with tile.TileContext(nc) as tc:
        with (
            tc.tile_pool(name="x1p", bufs=1) as x1p,
            tc.tile_pool(name="x2p", bufs=1) as x2p,
            tc.tile_pool(name="idxp", bufs=1) as idxp,
            tc.tile_pool(name="psp", bufs=8, space="PSUM") as psp,
            tc.tile_pool(name="bandp", bufs=len(CHUNKS)) as bandp,
            tc.tile_pool(name="xcp", bufs=len(CHUNKS)) as xcp,
        ):
            x1full = x1p.tile([C, S], BF16)
            x2full = x2p.tile([C, S + 2 * MAXD], BF16)
            idxt = idxp.tile([C, NIDX_COLS], U16)
            lo = 0
            for i, w in enumerate(SLICES):
                hi = lo + w
                nc.sync.dma_start(out=x1full[:, lo:hi], in_=x1[:, lo:hi])
                xhi = hi + 2 * MAXD if i == len(SLICES) - 1 else hi
                nc.sync.dma_start(out=x2full[:, lo:xhi], in_=x2[:, lo:xhi])
                if i == 0:
                    nc.sync.dma_start(out=idxt[:], in_=cidx[:])
                lo = hi

            idx_col_of = {}
            col = 0
            for kk, _ in sorted(set(CHUNKS)):
                idx_col_of[kk] = col
                col += _chunk_idx_cols(kk)

            t0 = 0
            for c, (K, grp) in enumerate(CHUNKS):
                ngrp = K // grp
                band = bandp.tile([C, K * NB], BF16)
                xct = xcp.tile([C, K * WIN], BF16)
                for g in range(ngrp):
                    ps = psp.tile([TP, grp * NB], F32)
                    for u in range(grp):
                        t = t0 + g * grp + u
                        nc.tensor.matmul(
                            ps[:, u * NB : (u + 1) * NB],
                            lhsT=x1full[:, TP * t : TP * (t + 1)],
                            rhs=x2full[:, TP * t : TP * t + NB],
                            start=True,
                            stop=True,
                        )
                    eng = nc.scalar if g % 2 == 0 else nc.vector
                    # grp-slots are laid out in REVERSE group order so the two
                    # last-written slots (one per copy engine) sit at the band
                    # base. The gather's declared data AP covers only that
                    # prefix: per-engine semaphore order then guarantees every
                    # earlier copy is also complete, while the (verified)
                    # base+index Q7 addressing reaches the full chunk band.
                    slot = ngrp - 1 - g
                    k0 = slot * grp
                    eng_copy = eng.copy if eng is nc.scalar else eng.tensor_copy
                    eng_copy(band[:, k0 * NB : (k0 + grp) * NB], ps[:])

                ic0 = idx_col_of[K]
                # data AP: all of slot 0 plus the first 8 cols of slot 1 — the
                # minimal contiguous span overlapping both engines' final
                # copies; indices address the rest of the band beyond it.
                span = grp * NB + 8 if ngrp > 1 else K * NB
                nc.gpsimd.indirect_copy(
                    out=xct[:].rearrange("p (i v) -> p i v", v=8),
                    data=band[:, 0:span].rearrange("p (e v) -> p e v", v=8),
                    idxs=idxt[:, ic0 : ic0 + _chunk_idx_cols(K)],
                    i_know_ap_gather_is_preferred=True,
                )
                nc.sync.dma_start(
                    out=xc[:, t0 * WIN : (t0 + K) * WIN], in_=xct[:]
                )
                t0 += K
    nc.compile()
    return nc


_NC_CACHE = {}


def _get_nc():
    if "nc" not in _NC_CACHE:
        _NC_CACHE["nc"] = _build_nc()
    return _NC_CACHE["nc"]


def _make_cidx() -> np.ndarray:
    """idx blocks for a chunk of K tiles: block i=(3k+u) gathers the 8 elems at
    window offset 8u of tile k for the core's partition group grp. Tile k lives
    in reversed GRP slot (K//GRP-1-k//GRP) at within-slot position k%GRP. Core
    grp reads its list wrapped across partitions 16grp..16grp+15: element i at
    [16grp + i%16, base_col + i//16]. Chunk sizes share one table, one column
    section per distinct K."""
    idx = np.zeros((C, NIDX_COLS), np.uint16)
    base = 0
    for kk, grp in sorted(set(CHUNKS)):
        ngrp = kk // grp
        for pg in range(8):
            for i in range(3 * kk):
                k, u = divmod(i, 3)
                col = (ngrp - 1 - k // grp) * grp * NB + (k % grp) * NB
                idx[16 * pg + i % 16, base + i // 16] = col + 16 * pg + 8 * u
        base += _chunk_idx_cols(kk)
    return idx


_CIDX = _make_cidx()
_PMOD = np.arange(C) % 16


def _extract(xc_core: np.ndarray) -> np.ndarray:
    """xc [C, NT*WIN] -> out [ND, H, W] with OOB zero mask applied."""
    v = np.asarray(xc_core).reshape(C, NT, WIN)
    y = np.empty((C, NT, ND), dtype=v.dtype)
    for r in range(16):
        y[r::16] = v[r::16, :, r : r + ND]
    out = np.ascontiguousarray(
        y.transpose(2, 1, 0).reshape(ND, H, W), dtype=np.float32
    )
    for j in range(ND):
        d = j - MAXD
        if d < 0:
            out[j, :, :-d] = 0.0
        elif d > 0:
            out[j, :, W - d :] = 0.0
    return out


def kernel(x1: np.ndarray, x2: np.ndarray) -> np.ndarray:
    assert x1.shape == (B, C, H, W) and x2.shape == (B, C, H, W)
    import ml_dtypes

    bf16 = ml_dtypes.bfloat16
    nc = _get_nc()
    # fold the 1/C mean scale into x1 (C = 128: exact exponent shift in bf16)
    x1b = (x1.reshape(B, C, S) * np.float32(1.0 / C)).astype(bf16)
    x2p = np.zeros((B, C, S + 2 * MAXD), dtype=bf16)
    x2p[:, :, MAXD : MAXD + S] = x2.reshape(B, C, S).astype(bf16)
    in_maps = [
        {"x1": np.ascontiguousarray(x1b[b]), "x2": x2p[b], "cidx": _CIDX}
        for b in range(B)
    ]

    trace = bool(int(os.environ.get("CORR_TRACE", "0")))
    res = bass_utils.run_bass_kernel_spmd(
        nc, in_maps, core_ids=list(range(B)), trace=trace
    )
    if trace:
        _NC_CACHE["last_results"] = res
    out = np.stack([_extract(res.results[b]["xc"]) for b in range(B)], axis=0)
    return out.astype(np.float32)
